# revision 14
# baseline (speedup 1.0000x reference)
"""Trainium2 Bass kernel: y = LP(square(BP(x))) cascaded-biquad IIR filtering.

x: [16, 16384, 64] fp32; bp_sos/lp_sos: [2, 6] second-order sections.
Reference applies, per (batch, channel) sequence along time:
  w = sosfilt(w, bp_sos); w = sosfilt(w*w, lp_sos)
with zero initial conditions (Direct Form I biquads).

Strategy (overlap-save FIR, no recurrence on device):
  Each 2-biquad cascade is an order-4 IIR whose impulse response h decays
  below 1e-12 within 256 samples (pole radii <= 0.84).  So the filter is,
  to fp16 precision, an FIR of 256 taps.  Chunking time into L=128 blocks
  (layout: partition = time-within-chunk, free = chunk*seq), each output
  chunk is EXACTLY two PE matmuls accumulated in PSUM:
     Y_c = A^T @ X_{c-1} + B^T @ X_c,   A[tau,t] = h[128+t-tau],
                                        B[tau,t] = h[t-tau] (t>=tau)
  i.e. pure feed-forward: no chunk-boundary state, no tail gathers, no
  sequential dependencies.  The first chunk reads a zeroed pad block
  (exact: zero initial conditions).

Per core: 128 sequences (rows of B*C=1024 split 8 ways), everything fp16
on device (inputs, weights, intermediates, output) with fp32 PSUM
accumulation; measured end-to-end error vs the float64 recurrence ~9e-4.

Engine budget per core (@ 2.4GHz PE, 1.2GHz Act, 0.96GHz DVE):
  PE:  2 filters x 2 passes x 16384 cols   = 65536 cy = 27.3 us
  Act: 16 squares  [128,1024] PSUM->SBUF   ~ 16 us
  DVE: 16 copies   [128,1024] PSUM->SBUF   ~ 19 us
  DMA: 4 MiB in + 4 MiB out @ ~360 GB/s    ~ 23 us
"""

import numpy as np

# ---------------------------------------------------------------- constants
B, T, C = 16, 16384, 64
NCORES = 8
L = 128           # chunk length == PE contraction depth
N = T // L        # 128 chunks per sequence
S = 128           # sequences per core
TILE = 512        # matmul moving free-dim (one PSUM bank of fp32)
PAIR = 2 * TILE   # consumer granularity (2 PSUM banks)
NP_ = (N * S) // PAIR  # 16 pairs per filter


def _combine_sos(sos):
    """[2,6] sos -> normalized order-4 (b[0..4], a[0..4]) float64, a[0]=1."""
    sos = np.asarray(sos, dtype=np.float64)
    b1, a1 = sos[0, :3] / sos[0, 3], sos[0, 3:] / sos[0, 3]
    b2, a2 = sos[1, :3] / sos[1, 3], sos[1, 3:] / sos[1, 3]
    return np.convolve(b1, b2), np.convolve(a1, a2)


def _impulse(b, a, n):
    """First n samples of the impulse response of the order-4 IIR (b, a)."""
    u = np.zeros(n + 4)
    y = np.zeros(n + 4)
    u[4] = 1.0
    h = np.zeros(n)
    for t in range(n):
        acc = b[0] * u[t + 4] + b[1] * u[t + 3] + b[2] * u[t + 2] \
            + b[3] * u[t + 1] + b[4] * u[t]
        acc -= a[1] * y[t + 3] + a[2] * y[t + 2] + a[3] * y[t + 1] + a[4] * y[t]
        y[t + 4] = acc
        h[t] = acc
    return h


def _fir_weights(sos):
    """(lhsT_list, n_passes): lhsT_j[tau, t] = h[j*L + t - tau] fp16.

    Pass j multiplies chunk c-j's inputs; j=0 is lower-triangular (causal).
    n_passes chosen so the discarded tail of h is < 1e-5 of peak.
    """
    bb, aa = _combine_sos(sos)
    h = _impulse(bb, aa, 6 * L)
    scale = np.abs(h).max()
    P = 2
    while P < 5 and np.abs(h[P * L:]).max() > 1e-5 * scale:
        P += 1
    if np.abs(h[P * L:]).max() > 1e-5 * scale:
        raise ValueError("impulse response does not decay within 5 chunks")
    idx = np.arange(L)
    K = idx[None, :] - idx[:, None]          # t - tau in [-127, 127]
    mats = []
    for j in range(P - 1, -1, -1):           # oldest history first
        M = h[np.clip(j * L + K, 0, 6 * L - 1)]
        if j == 0:
            M = np.where(K >= 0, M, 0.0)
        mats.append(np.ascontiguousarray(M.astype(np.float16)))
    return mats, P


# ---------------------------------------------------------------- device IR
_PROGRAM_CACHE = {}


def _build_program(p1, p2):
    """p1/p2: number of FIR passes for filter 1 / filter 2 (usually 2)."""
    import concourse.bass as bass
    import concourse.mybir as mybir
    import concourse.tile as tile
    from concourse import bacc

    F32 = mybir.dt.float32
    F16 = mybir.dt.float16
    ts = bass.ts
    PAD1 = (p1 - 1) * S   # zero-pad cols in front of filter-1 input
    PAD2 = (p2 - 1) * S

    nc = bacc.Bacc(None)
    x_d = nc.declare_dram_parameter("x", [128, T], F16, isOutput=False)
    w1_d = [nc.declare_dram_parameter(f"w1_{j}", [128, 128], F16, False)
            for j in range(p1)]
    w2_d = [nc.declare_dram_parameter(f"w2_{j}", [128, 128], F16, False)
            for j in range(p2)]
    out_d = nc.declare_dram_parameter("out", [128, T], F16, isOutput=True)

    with tile.TileContext(nc) as tc:
        with (
            tc.tile_pool(name="big", bufs=1) as bigpool,
            tc.tile_pool(name="consts", bufs=1) as cpool,
            tc.tile_pool(name="ps", bufs=4, space=bass.MemorySpace.PSUM) as pspool,
        ):
            xbuf = bigpool.tile([128, PAD1 + T], F16, tag="xbuf", name="xbuf")
            bufB = bigpool.tile([128, PAD2 + T], F16, tag="bufB", name="bufB")
            outsb = bigpool.tile([128, T], F16, tag="outsb", name="outsb")
            w1 = [cpool.tile([128, 128], F16, tag=f"w1{j}", name=f"w1{j}")
                  for j in range(p1)]
            w2 = [cpool.tile([128, 128], F16, tag=f"w2{j}", name=f"w2{j}")
                  for j in range(p2)]

            # zero pads (exact zero initial conditions)
            nc.vector.memzero(xbuf[:, 0:PAD1])
            nc.vector.memzero(bufB[:, 0:PAD2])
            # weights: issue from scalar queue (idle until first square)
            for j in range(p1):
                nc.scalar.dma_start(out=w1[j][:], in_=w1_d[j][:])
            for j in range(p2):
                nc.scalar.dma_start(out=w2[j][:], in_=w2_d[j][:])

            # input x: early pieces fan out across the Sync/Vector/GpSimd
            # queues so several transfers are in flight the moment the
            # framework preamble ends (a single queue issues serially and
            # starves the first matmuls).  Descriptors split to ~1KB each.
            def sdma(q, out_ap, in_ap):
                w = out_ap.shape[-1]
                k = 1
                for cand in (16, 8, 4, 2):
                    if w % cand == 0 and (w * 2) // cand >= 1024:
                        k = cand
                        break
                if k > 1:
                    out_ap = out_ap.rearrange("p (k w) -> p k w", k=k)
                    in_ap = in_ap.rearrange("p (k w) -> p k w", k=k)
                q.dma_start(out=out_ap, in_=in_ap)

            edges = [0, 512, 1536, 3584, 6656, 9728, 12800, T]
            queues = [nc.sync, nc.gpsimd]
            for i, (a, b) in enumerate(zip(edges[:-1], edges[1:])):
                sdma(queues[i % 2], xbuf[:, PAD1 + a: PAD1 + b], x_d[:, a:b])

            def emit_pair(IN, ws, p, consumer):
                """P passes x 2 matmuls (one per PSUM bank; >512 fp32 out cols
                per matmul fails the ISA check) -> one consumer op."""
                c0 = p * PAIR
                ps = pspool.tile([128, PAIR], F32, tag="ps", name="ps")
                for j in range(len(ws)):
                    off = c0 + j * S   # pass j reads chunk c-(npass-1-j)
                    first, last = (j == 0), (j == len(ws) - 1)
                    nc.tensor.matmul(ps[:, 0:TILE], ws[j][:],
                                     IN[:, off: off + TILE],
                                     start=first, stop=last,
                                     skip_group_check=True)
                    nc.tensor.matmul(ps[:, TILE:PAIR], ws[j][:],
                                     IN[:, off + TILE: off + PAIR],
                                     start=first, stop=last,
                                     skip_group_check=True)
                consumer(p, ps)

            # Filters interleaved on the PE at pair granularity with a small
            # lag.  Consumers: Act (activation square, ~1.0us/pair) drains
            # filter 1, DVE (cast copy, ~1.2us/pair) drains filter 2 -- each
            # engine sees one pair per TWO PE pair-slots (~1.7us), so PSUM
            # never backpressures the matmuls.  Trailing filter-2-only pairs
            # alternate Act/DVE (Act is free once filter 1 is done).
            def f1_consumer(p, ps):
                nc.scalar.square(
                    bufB[:, PAD2 + p * PAIR: PAD2 + (p + 1) * PAIR], ps[:])

            def f2_consumer(p, ps, eng):
                dst = outsb[:, ts(p, PAIR)]
                if eng == "v":
                    nc.vector.tensor_copy(dst, ps[:])
                else:
                    nc.scalar.copy(dst, ps[:])
                sdma(nc.sync, out_d[:, ts(p, PAIR)], outsb[:, ts(p, PAIR)])

            LAG = 3
            for p in range(NP_):
                emit_pair(xbuf, w1, p, f1_consumer)
                if p >= LAG:
                    emit_pair(bufB, w2, p - LAG,
                              lambda q, ps: f2_consumer(q, ps, "v"))
            for i, q in enumerate(range(NP_ - LAG, NP_)):
                emit_pair(bufB, w2, q,
                          lambda qq, ps, e=("s" if i % 2 == 0 else "v"):
                          f2_consumer(qq, ps, e))

    nc.compile()
    return nc


def _get_program(p1, p2):
    key = (p1, p2)
    if key not in _PROGRAM_CACHE:
        _PROGRAM_CACHE[key] = _build_program(p1, p2)
    return _PROGRAM_CACHE[key]


# ---------------------------------------------------------------- host entry
def _shard_inputs(x):
    """x [B,T,C] fp32 -> list of per-core Xm [128, T] fp16 arrays.

    Xm[l, c*S + s] = seq[core*S + s][c*L + l].
    """
    xs = np.ascontiguousarray(np.transpose(np.asarray(x, dtype=np.float32),
                                           (0, 2, 1))).reshape(B * C, T)
    xs = xs.astype(np.float16)
    shards = []
    for core in range(NCORES):
        seqs = xs[core * S: (core + 1) * S]
        Xm = np.ascontiguousarray(
            seqs.reshape(S, N, L).transpose(2, 1, 0)).reshape(L, N * S)
        shards.append(Xm)
    return shards


def _unshard_output(outs):
    """list of per-core [128, T] fp16 device outputs -> [B, T, C] fp32."""
    ys = np.empty((B * C, T), dtype=np.float32)
    for core in range(NCORES):
        O = np.asarray(outs[core]).astype(np.float32)
        ys[core * S: (core + 1) * S] = (
            O.reshape(L, N, S).transpose(2, 1, 0).reshape(S, T))
    return np.ascontiguousarray(ys.reshape(B, C, T).transpose(0, 2, 1))


def kernel(x, bp_sos, lp_sos, _trace=False, **_ignored):
    from concourse.bass_utils import run_bass_kernel_spmd

    w1, p1 = _fir_weights(np.asarray(bp_sos))
    w2, p2 = _fir_weights(np.asarray(lp_sos))
    consts = {f"w1_{j}": w1[j] for j in range(p1)}
    consts.update({f"w2_{j}": w2[j] for j in range(p2)})
    shards = _shard_inputs(x)
    nc = _get_program(p1, p2)
    in_maps = [dict(consts, x=shards[core]) for core in range(NCORES)]
    res = run_bass_kernel_spmd(nc, in_maps, list(range(NCORES)), trace=_trace)
    out = _unshard_output([res.results[core]["out"] for core in range(NCORES)])
    if _trace:
        return out, res
    return out


if __name__ == "__main__":
    rng = np.random.default_rng(0)
    x = rng.standard_normal((B, T, C), dtype=np.float32)
    sh = _shard_inputs(x)
    rt = _unshard_output(sh)
    print("roundtrip max err (fp16 quant only):",
          np.abs(rt - x).max())


# revision 18
# speedup vs baseline: 1.1806x; 1.1806x over previous
"""Trainium2 Bass kernel: y = LP(square(BP(x))) cascaded-biquad IIR filtering.

x: [16, 16384, 64] fp32; bp_sos/lp_sos: [2, 6] second-order sections.
Reference applies, per (batch, channel) sequence along time:
  w = sosfilt(w, bp_sos); w = sosfilt(w*w, lp_sos)
with zero initial conditions (Direct Form I biquads).

Strategy (overlap-save FIR, no recurrence on device):
  Each 2-biquad cascade is an order-4 IIR whose impulse response h decays
  below 1e-12 within 256 samples (pole radii <= 0.84).  So the filter is,
  to fp16 precision, an FIR of 256 taps.  Chunking time into L=128 blocks
  (layout: partition = time-within-chunk, free = chunk*seq), each output
  chunk is EXACTLY two PE matmuls accumulated in PSUM:
     Y_c = A^T @ X_{c-1} + B^T @ X_c,   A[tau,t] = h[128+t-tau],
                                        B[tau,t] = h[t-tau] (t>=tau)
  i.e. pure feed-forward: no chunk-boundary state, no tail gathers, no
  sequential dependencies.  The first chunk reads a zeroed pad block
  (exact: zero initial conditions).

Per core: 128 sequences (rows of B*C=1024 split 8 ways), everything fp16
on device (inputs, weights, intermediates, output) with fp32 PSUM
accumulation; measured end-to-end error vs the float64 recurrence ~9e-4.

Engine budget per core (@ 2.4GHz PE, 1.2GHz Act, 0.96GHz DVE):
  PE:  2 filters x 2 passes x 16384 cols   = 65536 cy = 27.3 us
  Act: 16 squares  [128,1024] PSUM->SBUF   ~ 16 us
  DVE: 16 copies   [128,1024] PSUM->SBUF   ~ 19 us
  DMA: 4 MiB in + 4 MiB out @ ~360 GB/s    ~ 23 us
"""

import numpy as np

# ---------------------------------------------------------------- constants
B, T, C = 16, 16384, 64
NCORES = 8
L = 128           # chunk length == PE contraction depth
N = T // L        # 128 chunks per sequence
S = 128           # sequences per core
TILE = 512        # matmul moving free-dim (one PSUM bank of fp32)
PAIR = 2 * TILE   # consumer granularity (2 PSUM banks)
NP_ = (N * S) // PAIR  # 16 pairs per filter


def _combine_sos(sos):
    """[2,6] sos -> normalized order-4 (b[0..4], a[0..4]) float64, a[0]=1."""
    sos = np.asarray(sos, dtype=np.float64)
    b1, a1 = sos[0, :3] / sos[0, 3], sos[0, 3:] / sos[0, 3]
    b2, a2 = sos[1, :3] / sos[1, 3], sos[1, 3:] / sos[1, 3]
    return np.convolve(b1, b2), np.convolve(a1, a2)


def _impulse(b, a, n):
    """First n samples of the impulse response of the order-4 IIR (b, a)."""
    u = np.zeros(n + 4)
    y = np.zeros(n + 4)
    u[4] = 1.0
    h = np.zeros(n)
    for t in range(n):
        acc = b[0] * u[t + 4] + b[1] * u[t + 3] + b[2] * u[t + 2] \
            + b[3] * u[t + 1] + b[4] * u[t]
        acc -= a[1] * y[t + 3] + a[2] * y[t + 2] + a[3] * y[t + 1] + a[4] * y[t]
        y[t + 4] = acc
        h[t] = acc
    return h


def _fir_weights(sos):
    """(lhsT_list, n_passes): lhsT_j[tau, t] = h[j*L + t - tau] fp16.

    Pass j multiplies chunk c-j's inputs; j=0 is lower-triangular (causal).
    n_passes chosen so the discarded tail of h is < 1e-5 of peak.
    """
    bb, aa = _combine_sos(sos)
    h = _impulse(bb, aa, 6 * L)
    scale = np.abs(h).max()
    P = 2
    while P < 5 and np.abs(h[P * L:]).max() > 1e-5 * scale:
        P += 1
    if np.abs(h[P * L:]).max() > 1e-5 * scale:
        raise ValueError("impulse response does not decay within 5 chunks")
    idx = np.arange(L)
    K = idx[None, :] - idx[:, None]          # t - tau in [-127, 127]
    mats = []
    for j in range(P - 1, -1, -1):           # oldest history first
        M = h[np.clip(j * L + K, 0, 6 * L - 1)]
        if j == 0:
            M = np.where(K >= 0, M, 0.0)
        mats.append(np.ascontiguousarray(M.astype(np.float16)))
    return mats, P


# ---------------------------------------------------------------- device IR
_PROGRAM_CACHE = {}


def _build_program(p1, p2):
    """p1/p2: number of FIR passes for filter 1 / filter 2 (usually 2)."""
    import concourse.bass as bass
    import concourse.mybir as mybir
    import concourse.tile as tile
    from concourse import bacc

    F32 = mybir.dt.float32
    F16 = mybir.dt.float16
    ts = bass.ts
    PAD1 = (p1 - 1) * S   # zero-pad cols in front of filter-1 input
    PAD2 = (p2 - 1) * S

    class FastExitTileContext(tile.TileContext):
        """TileContext whose exit skips the per-semaphore clear storm.

        The stock _drain_and_barrier emits a reset for every allocated
        semaphore (~250 instructions, ~7us serialized on the engine queues)
        before the final barrier.  Those clears are redundant for a one-shot
        kernel: Bass.__init__ already emits a full-range gpsimd semaphore
        clear in the program PROLOGUE, so the next execution starts clean
        regardless of the state this one leaves behind.
        """

        def _drain_and_barrier(self, tick_clock, wait_clock):
            from concourse.vector_clock import ScopedClock
            drain_inst = self.nc.sync.drain()
            wait_clock.add_sem_waits(
                drain_inst.ins, ScopedClock({None: tick_clock.global_clock})
            )
            self.nc.all_engine_barrier()
            popped = self.nc._tile_sem_poison_stack.pop()
            assert popped is self._sem_poison
            assert self.sems is not None
            # Free the handles for bookkeeping but emit no clears.
            for handle in self.sems.allocated().values():
                self.nc.release_semaphore(handle)

    nc = bacc.Bacc(None)
    x_d = nc.declare_dram_parameter("x", [128, T], F16, isOutput=False)
    w1_d = [nc.declare_dram_parameter(f"w1_{j}", [128, 128], F16, False)
            for j in range(p1)]
    w2_d = [nc.declare_dram_parameter(f"w2_{j}", [128, 128], F16, False)
            for j in range(p2)]
    out_d = nc.declare_dram_parameter("out", [128, T], F16, isOutput=True)

    with FastExitTileContext(nc) as tc:
        with (
            tc.tile_pool(name="big", bufs=1) as bigpool,
            tc.tile_pool(name="consts", bufs=1) as cpool,
            tc.tile_pool(name="ps", bufs=4, space=bass.MemorySpace.PSUM) as pspool,
        ):
            xbuf = bigpool.tile([128, PAD1 + T], F16, tag="xbuf", name="xbuf")
            bufB = bigpool.tile([128, PAD2 + T], F16, tag="bufB", name="bufB")
            outsb = bigpool.tile([128, T], F16, tag="outsb", name="outsb")
            w1 = [cpool.tile([128, 128], F16, tag=f"w1{j}", name=f"w1{j}")
                  for j in range(p1)]
            w2 = [cpool.tile([128, 128], F16, tag=f"w2{j}", name=f"w2{j}")
                  for j in range(p2)]

            # zero pads (exact zero initial conditions)
            nc.vector.memzero(xbuf[:, 0:PAD1])
            nc.vector.memzero(bufB[:, 0:PAD2])
            # weights: issue from scalar queue (idle until first square)
            for j in range(p1):
                nc.scalar.dma_start(out=w1[j][:], in_=w1_d[j][:])
            for j in range(p2):
                nc.scalar.dma_start(out=w2[j][:], in_=w2_d[j][:])

            # input x: early pieces fan out across the Sync/Vector/GpSimd
            # queues so several transfers are in flight the moment the
            # framework preamble ends (a single queue issues serially and
            # starves the first matmuls).  Descriptors split to ~1KB each.
            def sdma(q, out_ap, in_ap):
                w = out_ap.shape[-1]
                k = 1
                for cand in (16, 8, 4, 2):
                    if w % cand == 0 and (w * 2) // cand >= 1024:
                        k = cand
                        break
                if k > 1:
                    out_ap = out_ap.rearrange("p (k w) -> p k w", k=k)
                    in_ap = in_ap.rearrange("p (k w) -> p k w", k=k)
                q.dma_start(out=out_ap, in_=in_ap)

            edges = [0, 512, 1536, 3072, 6144, 9216, 12288, T]
            for a, b in zip(edges[:-1], edges[1:]):
                sdma(nc.sync, xbuf[:, PAD1 + a: PAD1 + b], x_d[:, a:b])

            def emit_pair(IN, ws, p, consumer):
                """P passes x 2 matmuls (one per PSUM bank; >512 fp32 out cols
                per matmul fails the ISA check) -> one consumer op."""
                c0 = p * PAIR
                ps = pspool.tile([128, PAIR], F32, tag="ps", name="ps")
                for j in range(len(ws)):
                    off = c0 + j * S   # pass j reads chunk c-(npass-1-j)
                    first, last = (j == 0), (j == len(ws) - 1)
                    nc.tensor.matmul(ps[:, 0:TILE], ws[j][:],
                                     IN[:, off: off + TILE],
                                     start=first, stop=last,
                                     skip_group_check=True)
                    nc.tensor.matmul(ps[:, TILE:PAIR], ws[j][:],
                                     IN[:, off + TILE: off + PAIR],
                                     start=first, stop=last,
                                     skip_group_check=True)
                consumer(p, ps)

            # Filters interleaved on the PE at pair granularity with a small
            # lag.  Consumers: Act (activation square, ~1.0us/pair) drains
            # filter 1, DVE (cast copy, ~1.2us/pair) drains filter 2 -- each
            # engine sees one pair per TWO PE pair-slots (~1.7us), so PSUM
            # never backpressures the matmuls.  Trailing filter-2-only pairs
            # alternate Act/DVE (Act is free once filter 1 is done).
            def f1_consumer(p, ps):
                nc.scalar.square(
                    bufB[:, PAD2 + p * PAIR: PAD2 + (p + 1) * PAIR], ps[:])

            def f2_consumer(p, ps, eng):
                dst = outsb[:, ts(p, PAIR)]
                if eng == "v":
                    nc.vector.tensor_copy(dst, ps[:])
                else:
                    nc.scalar.copy(dst, ps[:])
                sdma(nc.sync, out_d[:, ts(p, PAIR)], outsb[:, ts(p, PAIR)])

            LAG = 2
            for p in range(NP_):
                emit_pair(xbuf, w1, p, f1_consumer)
                if p >= LAG:
                    emit_pair(bufB, w2, p - LAG,
                              lambda q, ps: f2_consumer(q, ps, "v"))
            for i, q in enumerate(range(NP_ - LAG, NP_)):
                emit_pair(bufB, w2, q,
                          lambda qq, ps, e=("s" if i % 2 == 0 else "v"):
                          f2_consumer(qq, ps, e))

    nc.compile()
    return nc


def _get_program(p1, p2):
    key = (p1, p2)
    if key not in _PROGRAM_CACHE:
        _PROGRAM_CACHE[key] = _build_program(p1, p2)
    return _PROGRAM_CACHE[key]


# ---------------------------------------------------------------- host entry
def _shard_inputs(x):
    """x [B,T,C] fp32 -> list of per-core Xm [128, T] fp16 arrays.

    Xm[l, c*S + s] = seq[core*S + s][c*L + l].
    """
    xs = np.ascontiguousarray(np.transpose(np.asarray(x, dtype=np.float32),
                                           (0, 2, 1))).reshape(B * C, T)
    xs = xs.astype(np.float16)
    shards = []
    for core in range(NCORES):
        seqs = xs[core * S: (core + 1) * S]
        Xm = np.ascontiguousarray(
            seqs.reshape(S, N, L).transpose(2, 1, 0)).reshape(L, N * S)
        shards.append(Xm)
    return shards


def _unshard_output(outs):
    """list of per-core [128, T] fp16 device outputs -> [B, T, C] fp32."""
    ys = np.empty((B * C, T), dtype=np.float32)
    for core in range(NCORES):
        O = np.asarray(outs[core]).astype(np.float32)
        ys[core * S: (core + 1) * S] = (
            O.reshape(L, N, S).transpose(2, 1, 0).reshape(S, T))
    return np.ascontiguousarray(ys.reshape(B, C, T).transpose(0, 2, 1))


def kernel(x, bp_sos, lp_sos, _trace=False, **_ignored):
    from concourse.bass_utils import run_bass_kernel_spmd

    w1, p1 = _fir_weights(np.asarray(bp_sos))
    w2, p2 = _fir_weights(np.asarray(lp_sos))
    consts = {f"w1_{j}": w1[j] for j in range(p1)}
    consts.update({f"w2_{j}": w2[j] for j in range(p2)})
    shards = _shard_inputs(x)
    nc = _get_program(p1, p2)
    in_maps = [dict(consts, x=shards[core]) for core in range(NCORES)]
    res = run_bass_kernel_spmd(nc, in_maps, list(range(NCORES)), trace=_trace)
    out = _unshard_output([res.results[core]["out"] for core in range(NCORES)])
    if _trace:
        return out, res
    return out


if __name__ == "__main__":
    rng = np.random.default_rng(0)
    x = rng.standard_normal((B, T, C), dtype=np.float32)
    sh = _shard_inputs(x)
    rt = _unshard_output(sh)
    print("roundtrip max err (fp16 quant only):",
          np.abs(rt - x).max())


# revision 20
# speedup vs baseline: 1.1952x; 1.0123x over previous
"""Trainium2 Bass kernel: y = LP(square(BP(x))) cascaded-biquad IIR filtering.

x: [16, 16384, 64] fp32; bp_sos/lp_sos: [2, 6] second-order sections.
Reference applies, per (batch, channel) sequence along time:
  w = sosfilt(w, bp_sos); w = sosfilt(w*w, lp_sos)
with zero initial conditions (Direct Form I biquads).

Strategy (overlap-save FIR, no recurrence on device):
  Each 2-biquad cascade is an order-4 IIR whose impulse response h decays
  below 1e-12 within 256 samples (pole radii <= 0.84).  So the filter is,
  to fp16 precision, an FIR of 256 taps.  Chunking time into L=128 blocks
  (layout: partition = time-within-chunk, free = chunk*seq), each output
  chunk is EXACTLY two PE matmuls accumulated in PSUM:
     Y_c = A^T @ X_{c-1} + B^T @ X_c,   A[tau,t] = h[128+t-tau],
                                        B[tau,t] = h[t-tau] (t>=tau)
  i.e. pure feed-forward: no chunk-boundary state, no tail gathers, no
  sequential dependencies.  The first chunk reads a zeroed pad block
  (exact: zero initial conditions).

Per core: 128 sequences (rows of B*C=1024 split 8 ways), everything fp16
on device (inputs, weights, intermediates, output) with fp32 PSUM
accumulation; measured end-to-end error vs the float64 recurrence ~9e-4.

Engine budget per core (@ 2.4GHz PE, 1.2GHz Act, 0.96GHz DVE):
  PE:  2 filters x 2 passes x 16384 cols   = 65536 cy = 27.3 us
  Act: 16 squares  [128,1024] PSUM->SBUF   ~ 16 us
  DVE: 16 copies   [128,1024] PSUM->SBUF   ~ 19 us
  DMA: 4 MiB in + 4 MiB out @ ~360 GB/s    ~ 23 us
"""

import numpy as np

# ---------------------------------------------------------------- constants
B, T, C = 16, 16384, 64
NCORES = 8
L = 128           # chunk length == PE contraction depth
N = T // L        # 128 chunks per sequence
S = 128           # sequences per core
TILE = 512        # matmul moving free-dim (one PSUM bank of fp32)
PAIR = 2 * TILE   # consumer granularity (2 PSUM banks)
NP_ = (N * S) // PAIR  # 16 pairs per filter


def _combine_sos(sos):
    """[2,6] sos -> normalized order-4 (b[0..4], a[0..4]) float64, a[0]=1."""
    sos = np.asarray(sos, dtype=np.float64)
    b1, a1 = sos[0, :3] / sos[0, 3], sos[0, 3:] / sos[0, 3]
    b2, a2 = sos[1, :3] / sos[1, 3], sos[1, 3:] / sos[1, 3]
    return np.convolve(b1, b2), np.convolve(a1, a2)


def _impulse(b, a, n):
    """First n samples of the impulse response of the order-4 IIR (b, a)."""
    u = np.zeros(n + 4)
    y = np.zeros(n + 4)
    u[4] = 1.0
    h = np.zeros(n)
    for t in range(n):
        acc = b[0] * u[t + 4] + b[1] * u[t + 3] + b[2] * u[t + 2] \
            + b[3] * u[t + 1] + b[4] * u[t]
        acc -= a[1] * y[t + 3] + a[2] * y[t + 2] + a[3] * y[t + 1] + a[4] * y[t]
        y[t + 4] = acc
        h[t] = acc
    return h


def _fir_weights(sos):
    """(lhsT_list, n_passes): lhsT_j[tau, t] = h[j*L + t - tau] fp16.

    Pass j multiplies chunk c-j's inputs; j=0 is lower-triangular (causal).
    n_passes chosen so the discarded tail of h is < 1e-5 of peak.
    """
    bb, aa = _combine_sos(sos)
    h = _impulse(bb, aa, 6 * L)
    scale = np.abs(h).max()
    P = 2
    while P < 5 and np.abs(h[P * L:]).max() > 1e-5 * scale:
        P += 1
    if np.abs(h[P * L:]).max() > 1e-5 * scale:
        raise ValueError("impulse response does not decay within 5 chunks")
    idx = np.arange(L)
    K = idx[None, :] - idx[:, None]          # t - tau in [-127, 127]
    mats = []
    for j in range(P - 1, -1, -1):           # oldest history first
        M = h[np.clip(j * L + K, 0, 6 * L - 1)]
        if j == 0:
            M = np.where(K >= 0, M, 0.0)
        mats.append(np.ascontiguousarray(M.astype(np.float16)))
    return mats, P


# ---------------------------------------------------------------- device IR
_PROGRAM_CACHE = {}


def _build_program(p1, p2):
    """p1/p2: number of FIR passes for filter 1 / filter 2 (usually 2)."""
    import concourse.bass as bass
    import concourse.mybir as mybir
    import concourse.tile as tile
    from concourse import bacc

    F32 = mybir.dt.float32
    F16 = mybir.dt.float16
    ts = bass.ts
    PAD1 = (p1 - 1) * S   # zero-pad cols in front of filter-1 input
    PAD2 = (p2 - 1) * S

    class FastExitTileContext(tile.TileContext):
        """TileContext whose exit skips the per-semaphore clear storm.

        The stock _drain_and_barrier emits a reset for every allocated
        semaphore (~250 instructions, ~7us serialized on the engine queues)
        before the final barrier.  Those clears are redundant for a one-shot
        kernel: Bass.__init__ already emits a full-range gpsimd semaphore
        clear in the program PROLOGUE, so the next execution starts clean
        regardless of the state this one leaves behind.
        """

        def _drain_and_barrier(self, tick_clock, wait_clock):
            from concourse.vector_clock import ScopedClock
            drain_inst = self.nc.sync.drain()
            wait_clock.add_sem_waits(
                drain_inst.ins, ScopedClock({None: tick_clock.global_clock})
            )
            self.nc.all_engine_barrier()
            popped = self.nc._tile_sem_poison_stack.pop()
            assert popped is self._sem_poison
            assert self.sems is not None
            # Free the handles for bookkeeping but emit no clears.
            for handle in self.sems.allocated().values():
                self.nc.release_semaphore(handle)

    nc = bacc.Bacc(None)
    x_d = nc.declare_dram_parameter("x", [128, T], F16, isOutput=False)
    w1_d = [nc.declare_dram_parameter(f"w1_{j}", [128, 128], F16, False)
            for j in range(p1)]
    w2_d = [nc.declare_dram_parameter(f"w2_{j}", [128, 128], F16, False)
            for j in range(p2)]
    out_d = nc.declare_dram_parameter("out", [128, T], F16, isOutput=True)

    with FastExitTileContext(nc) as tc:
        with (
            tc.tile_pool(name="big", bufs=1) as bigpool,
            tc.tile_pool(name="consts", bufs=1) as cpool,
            tc.tile_pool(name="ps", bufs=4, space=bass.MemorySpace.PSUM) as pspool,
        ):
            xbuf = bigpool.tile([128, PAD1 + T], F16, tag="xbuf", name="xbuf")
            bufB = bigpool.tile([128, PAD2 + T], F16, tag="bufB", name="bufB")
            outsb = bigpool.tile([128, T], F16, tag="outsb", name="outsb")
            w1 = [cpool.tile([128, 128], F16, tag=f"w1{j}", name=f"w1{j}")
                  for j in range(p1)]
            w2 = [cpool.tile([128, 128], F16, tag=f"w2{j}", name=f"w2{j}")
                  for j in range(p2)]

            # zero pads (exact zero initial conditions)
            nc.vector.memzero(xbuf[:, 0:PAD1])
            nc.vector.memzero(bufB[:, 0:PAD2])
            # first-needed weight leads the sync queue (gates matmul 0);
            # the rest issue from the scalar queue in parallel.
            nc.sync.dma_start(out=w1[0][:], in_=w1_d[0][:])
            for j in range(1, p1):
                nc.scalar.dma_start(out=w1[j][:], in_=w1_d[j][:])
            for j in range(p2):
                nc.scalar.dma_start(out=w2[j][:], in_=w2_d[j][:])

            # input x: early pieces fan out across the Sync/Vector/GpSimd
            # queues so several transfers are in flight the moment the
            # framework preamble ends (a single queue issues serially and
            # starves the first matmuls).  Descriptors split to ~1KB each.
            def sdma(q, out_ap, in_ap):
                w = out_ap.shape[-1]
                k = 1
                for cand in (16, 8, 4, 2):
                    if w % cand == 0 and (w * 2) // cand >= 1024:
                        k = cand
                        break
                if k > 1:
                    out_ap = out_ap.rearrange("p (k w) -> p k w", k=k)
                    in_ap = in_ap.rearrange("p (k w) -> p k w", k=k)
                q.dma_start(out=out_ap, in_=in_ap)

            for a in range(0, T, 1024):
                sdma(nc.sync, xbuf[:, PAD1 + a: PAD1 + a + 1024],
                     x_d[:, a: a + 1024])

            def emit_pair(IN, ws, p, consumer):
                """P passes x 2 matmuls (one per PSUM bank; >512 fp32 out cols
                per matmul fails the ISA check) -> one consumer op."""
                c0 = p * PAIR
                ps = pspool.tile([128, PAIR], F32, tag="ps", name="ps")
                for j in range(len(ws)):
                    off = c0 + j * S   # pass j reads chunk c-(npass-1-j)
                    first, last = (j == 0), (j == len(ws) - 1)
                    nc.tensor.matmul(ps[:, 0:TILE], ws[j][:],
                                     IN[:, off: off + TILE],
                                     start=first, stop=last,
                                     skip_group_check=True)
                    nc.tensor.matmul(ps[:, TILE:PAIR], ws[j][:],
                                     IN[:, off + TILE: off + PAIR],
                                     start=first, stop=last,
                                     skip_group_check=True)
                consumer(p, ps)

            # Filters interleaved on the PE at pair granularity with a small
            # lag.  Consumers: Act (activation square, ~1.0us/pair) drains
            # filter 1, DVE (cast copy, ~1.2us/pair) drains filter 2 -- each
            # engine sees one pair per TWO PE pair-slots (~1.7us), so PSUM
            # never backpressures the matmuls.  Trailing filter-2-only pairs
            # alternate Act/DVE (Act is free once filter 1 is done).
            def f1_consumer(p, ps):
                nc.scalar.square(
                    bufB[:, PAD2 + p * PAIR: PAD2 + (p + 1) * PAIR], ps[:])

            def f2_consumer(p, ps, eng):
                dst = outsb[:, ts(p, PAIR)]
                if eng == "v":
                    nc.vector.tensor_copy(dst, ps[:])
                else:
                    nc.scalar.copy(dst, ps[:])
                sdma(nc.sync, out_d[:, ts(p, PAIR)], outsb[:, ts(p, PAIR)])

            LAG = 2
            for p in range(NP_):
                emit_pair(xbuf, w1, p, f1_consumer)
                if p >= LAG:
                    emit_pair(bufB, w2, p - LAG,
                              lambda q, ps: f2_consumer(q, ps, "v"))
            for i, q in enumerate(range(NP_ - LAG, NP_)):
                emit_pair(bufB, w2, q,
                          lambda qq, ps, e=("s" if i % 2 == 0 else "v"):
                          f2_consumer(qq, ps, e))

    nc.compile()
    return nc


def _get_program(p1, p2):
    key = (p1, p2)
    if key not in _PROGRAM_CACHE:
        _PROGRAM_CACHE[key] = _build_program(p1, p2)
    return _PROGRAM_CACHE[key]


# ---------------------------------------------------------------- host entry
def _shard_inputs(x):
    """x [B,T,C] fp32 -> list of per-core Xm [128, T] fp16 arrays.

    Xm[l, c*S + s] = seq[core*S + s][c*L + l].
    """
    xs = np.ascontiguousarray(np.transpose(np.asarray(x, dtype=np.float32),
                                           (0, 2, 1))).reshape(B * C, T)
    xs = xs.astype(np.float16)
    shards = []
    for core in range(NCORES):
        seqs = xs[core * S: (core + 1) * S]
        Xm = np.ascontiguousarray(
            seqs.reshape(S, N, L).transpose(2, 1, 0)).reshape(L, N * S)
        shards.append(Xm)
    return shards


def _unshard_output(outs):
    """list of per-core [128, T] fp16 device outputs -> [B, T, C] fp32."""
    ys = np.empty((B * C, T), dtype=np.float32)
    for core in range(NCORES):
        O = np.asarray(outs[core]).astype(np.float32)
        ys[core * S: (core + 1) * S] = (
            O.reshape(L, N, S).transpose(2, 1, 0).reshape(S, T))
    return np.ascontiguousarray(ys.reshape(B, C, T).transpose(0, 2, 1))


def kernel(x, bp_sos, lp_sos, _trace=False, **_ignored):
    from concourse.bass_utils import run_bass_kernel_spmd

    w1, p1 = _fir_weights(np.asarray(bp_sos))
    w2, p2 = _fir_weights(np.asarray(lp_sos))
    consts = {f"w1_{j}": w1[j] for j in range(p1)}
    consts.update({f"w2_{j}": w2[j] for j in range(p2)})
    shards = _shard_inputs(x)
    nc = _get_program(p1, p2)
    in_maps = [dict(consts, x=shards[core]) for core in range(NCORES)]
    res = run_bass_kernel_spmd(nc, in_maps, list(range(NCORES)), trace=_trace)
    out = _unshard_output([res.results[core]["out"] for core in range(NCORES)])
    if _trace:
        return out, res
    return out


if __name__ == "__main__":
    rng = np.random.default_rng(0)
    x = rng.standard_normal((B, T, C), dtype=np.float32)
    sh = _shard_inputs(x)
    rt = _unshard_output(sh)
    print("roundtrip max err (fp16 quant only):",
          np.abs(rt - x).max())
